# revision 1
# baseline (speedup 1.0000x reference)
"""CLUB-NCE loss kernel for 8x Trainium2 NeuronCores (Bass/Tile).

Math (reference):
  hx = x @ W1x.T, hy = y @ W1y.T            [N, H]
  s[i,j]  = W2 . relu(hy[i] + hx[j] + b1) + b2
  T1[i,j] = softplus(s[i,j]); T0[i] = T1[i,i]
  lower = mean(T0) - (mean_i(logsumexp_j(T1[i,:])) - log N)
  upper = mean(T0) - mean(T1)

Sharding: y rows (i axis) split across 8 cores (64 rows each); x and MLP
params replicated. Each core computes its [64, 512] score block, converts
rows to exp-space (exp(softplus(s)) = 1 + e^s, so logsumexp over a row is
log(512 + sum_j e^s) with no max pass needed), and emits per-row partials
(row lse, row sum of T1, diag element). Host combines the scalar partials.

Device layout: contraction dim k (=H, padded 400->512) on partitions.
  hxT   [512k, 512j] fp16 (4 tiles of [128, 512])
  hybT  [512k,  64i] f32  (hy + b1, transposed)
  per i: r[kt] = fp16(relu(hxT[kt] + hybT[kt][:, i]))   (DVE 4x mode)
         psum[1, 512] += w2[kt].T @ r[kt]               (PE, fp16)
         E row = exp(psum + b2)                         (ACT, drains psum)
"""

import numpy as np

N = 512          # number of samples
D = 400          # feature dim
H = 400          # hidden dim
NCORES = 8
NL = N // NCORES  # 64 y-rows per core
KP = 512          # padded contraction dim
KT = 4            # 128-partition k tiles


def _build_program(b2val: float, enable_asserts: bool = False):
    import concourse.bacc as bacc
    import concourse.mybir as mybir
    import concourse.tile as tile

    fp16 = mybir.dt.float16
    f32 = mybir.dt.float32
    AF = mybir.ActivationFunctionType
    ALU = mybir.AluOpType

    nc = bacc.Bacc(
        "TRN2",
        target_bir_lowering=False,
        debug=False,
        enable_asserts=enable_asserts,
    )

    xT = nc.dram_tensor("xT", [KP, N], fp16, kind="ExternalInput")
    w1xT = nc.dram_tensor("w1xT", [KP, KP], fp16, kind="ExternalInput")
    w1yT = nc.dram_tensor("w1yT", [KP, KP], fp16, kind="ExternalInput")
    yT = nc.dram_tensor("yT", [KP, NL], fp16, kind="ExternalInput")
    b1c = nc.dram_tensor("b1c", [KP, 1], f32, kind="ExternalInput")
    w2c = nc.dram_tensor("w2c", [KP, 1], fp16, kind="ExternalInput")
    maskd = nc.dram_tensor("maskd", [NL, N], f32, kind="ExternalInput")

    lse_o = nc.dram_tensor("lse_o", [1, NL], f32, kind="ExternalOutput")
    rs_o = nc.dram_tensor("rs_o", [NL, 1], f32, kind="ExternalOutput")
    t0_o = nc.dram_tensor("t0_o", [NL, 1], f32, kind="ExternalOutput")

    eflat_d = nc.dram_tensor("eflat_d", [1, NL * N], f32)  # bounce buffer

    with tile.TileContext(nc) as tc:
        with (
            tc.tile_pool(name="const", bufs=1) as cpool,
            tc.tile_pool(name="work", bufs=32) as wpool,
            tc.tile_pool(name="ppro", bufs=2, space="PSUM") as ppro,
            tc.tile_pool(name="pmain", bufs=6, space="PSUM") as pmain,
        ):
            xt, w1x, w1y, yt, b1t, w2t = [], [], [], [], [], []
            for k in range(KT):
                sl = slice(k * 128, (k + 1) * 128)
                t = cpool.tile([128, N], fp16, name=f"xt{k}")
                nc.sync.dma_start(out=t, in_=xT[sl, :])
                xt.append(t)
                t = cpool.tile([128, KP], fp16, name=f"w1x{k}")
                nc.sync.dma_start(out=t, in_=w1xT[sl, :])
                w1x.append(t)
                t = cpool.tile([128, KP], fp16, name=f"w1y{k}")
                nc.sync.dma_start(out=t, in_=w1yT[sl, :])
                w1y.append(t)
                t = cpool.tile([128, NL], fp16, name=f"yt{k}")
                nc.sync.dma_start(out=t, in_=yT[sl, :])
                yt.append(t)
                t = cpool.tile([128, 1], f32, name=f"b1t{k}")
                nc.sync.dma_start(out=t, in_=b1c[sl, :])
                b1t.append(t)
                t = cpool.tile([128, 1], fp16, name=f"w2t{k}")
                nc.sync.dma_start(out=t, in_=w2c[sl, :])
                w2t.append(t)
            mask = cpool.tile([NL, N], f32, name="mask")
            nc.sync.dma_start(out=mask, in_=maskd[:, :])
            b2t = cpool.tile([1, 1], f32, name="b2t")
            nc.vector.memset(b2t, b2val)
            n512t = cpool.tile([1, 1], f32, name="n512t")
            nc.vector.memset(n512t, float(N))

            # ---- prologue: hxT (fp16) and hybT (f32) ----
            hx, hyb = [], []
            for m in range(KT):
                msl = slice(m * 128, (m + 1) * 128)
                ph = ppro.tile([128, N], f32, name=f"ph{m}", tag="pp")
                for k in range(KT):
                    nc.tensor.matmul(
                        ph, lhsT=w1x[k][:, msl], rhs=xt[k],
                        start=(k == 0), stop=(k == KT - 1),
                    )
                hxm = cpool.tile([128, N], fp16, name=f"hx{m}")
                nc.vector.tensor_copy(out=hxm, in_=ph)
                hx.append(hxm)
            for m in range(KT):
                msl = slice(m * 128, (m + 1) * 128)
                py = ppro.tile([128, NL], f32, name=f"py{m}", tag="pp")
                for k in range(KT):
                    nc.tensor.matmul(
                        py, lhsT=w1y[k][:, msl], rhs=yt[k],
                        start=(k == 0), stop=(k == KT - 1),
                    )
                hybm = cpool.tile([128, NL], f32, name=f"hyb{m}")
                nc.vector.tensor_scalar_add(hybm, py, b1t[m])
                hyb.append(hybm)

            # ---- main loop over local y rows ----
            eflat = cpool.tile([1, NL * N], f32, name="eflat")
            rrow = cpool.tile([1, NL], f32, name="rrow")
            for i in range(NL):
                ps = pmain.tile([1, N], f32, name="ps", tag="ps")
                for k in range(KT):
                    r = wpool.tile([128, N], fp16, name="r", tag="r")
                    nc.vector.tensor_scalar(
                        out=r, in0=hx[k],
                        scalar1=hyb[k][:, i : i + 1], scalar2=0.0,
                        op0=ALU.add, op1=ALU.max,
                    )
                    nc.tensor.matmul(
                        ps, lhsT=w2t[k], rhs=r,
                        start=(k == 0), stop=(k == KT - 1),
                    )
                # drain psum row: E = exp(s + b2), R[i] = sum_j E
                nc.scalar.activation(
                    out=eflat[:, i * N : (i + 1) * N], in_=ps,
                    func=AF.Exp, bias=b2t[0:1, :], scale=1.0,
                    accum_out=rrow[:, i : i + 1],
                )

            # ---- restructure E rows [1, NL*N] -> [NL, N] via DRAM bounce ----
            nc.sync.dma_start(out=eflat_d[:, :], in_=eflat)
            e2 = cpool.tile([NL, N], f32, name="e2")
            nc.sync.dma_start(
                out=e2, in_=eflat_d.ap().rearrange("o (i j) -> (o i) j", i=NL)
            )

            # ---- postprocessing ----
            t1 = cpool.tile([NL, N], f32, name="t1")
            rs = cpool.tile([NL, 1], f32, name="rs")
            # T1 = log(1 + E) = softplus(s); rs = row sums of T1
            nc.scalar.activation(
                out=t1, in_=e2, func=AF.Ln, bias=1.0, scale=1.0
            )
            nc.vector.reduce_sum(out=rs, in_=t1, axis=mybir.AxisListType.X)
            lse = cpool.tile([1, NL], f32, name="lse")
            # row logsumexp = log(512 + sum_j e^s)
            nc.scalar.activation(
                out=lse, in_=rrow, func=AF.Ln, bias=n512t[0:1, :], scale=1.0
            )
            junk = cpool.tile([NL, N], f32, name="junk")
            t0 = cpool.tile([NL, 1], f32, name="t0")
            nc.vector.tensor_tensor(
                out=junk, in0=t1, in1=mask, op=ALU.mult
            )
            nc.vector.reduce_sum(out=t0, in_=junk, axis=mybir.AxisListType.X)
            nc.sync.dma_start(out=lse_o[:, :], in_=lse)
            nc.sync.dma_start(out=rs_o[:, :], in_=rs)
            nc.sync.dma_start(out=t0_o[:, :], in_=t0)

    nc.compile()
    return nc


def _make_in_maps(x, y, W1, b1, W2):
    f16 = np.float16
    xTp = np.zeros((KP, N), f16)
    xTp[:D, :] = x.T.astype(f16)
    w1xTp = np.zeros((KP, KP), f16)
    w1xTp[:D, :H] = W1[:, :D].T.astype(f16)
    w1yTp = np.zeros((KP, KP), f16)
    w1yTp[:D, :H] = W1[:, D:].T.astype(f16)
    b1p = np.zeros((KP, 1), np.float32)
    b1p[:H, 0] = b1
    w2p = np.zeros((KP, 1), f16)
    w2p[:H, 0] = W2[0].astype(f16)

    in_maps = []
    for c in range(NCORES):
        yTp = np.zeros((KP, NL), f16)
        yTp[:D, :] = y[c * NL : (c + 1) * NL, :].T.astype(f16)
        mask = np.zeros((NL, N), np.float32)
        mask[np.arange(NL), c * NL + np.arange(NL)] = 1.0
        in_maps.append(
            {
                "xT": xTp, "w1xT": w1xTp, "w1yT": w1yTp, "yT": yTp,
                "b1c": b1p, "w2c": w2p, "maskd": mask,
            }
        )
    return in_maps


def _combine(results):
    lse_all = np.concatenate([r["lse_o"][0].astype(np.float64) for r in results])
    rs_all = np.concatenate([r["rs_o"][:, 0].astype(np.float64) for r in results])
    t0_all = np.concatenate([r["t0_o"][:, 0].astype(np.float64) for r in results])
    t0_mean = t0_all.mean()
    lower = t0_mean - (lse_all.mean() - np.log(np.float64(N)))
    upper = t0_mean - rs_all.mean() / N
    return np.float32(lower), np.float32(upper)


def kernel(x_samples, y_samples, W1, b1, W2, b2, _trace=False):
    from concourse.bass_utils import run_bass_kernel_spmd

    nc = _build_program(float(np.float32(b2[0])))
    in_maps = _make_in_maps(
        np.asarray(x_samples, np.float32),
        np.asarray(y_samples, np.float32),
        np.asarray(W1, np.float32),
        np.asarray(b1, np.float32),
        np.asarray(W2, np.float32),
    )
    res = run_bass_kernel_spmd(
        nc, in_maps, core_ids=list(range(NCORES)), trace=_trace
    )
    out = _combine(res.results)
    if _trace:
        return out, res
    return out



# revision 6
# speedup vs baseline: 1.1202x; 1.1202x over previous
"""CLUB-NCE loss kernel for 8x Trainium2 NeuronCores (Bass/Tile).

Math (reference):
  hx = x @ W1x.T, hy = y @ W1y.T            [N, H]
  s[i,j]  = W2 . relu(hy[i] + hx[j] + b1) + b2
  T1[i,j] = softplus(s[i,j]); T0[i] = T1[i,i]
  lower = mean(T0) - (mean_i(logsumexp_j(T1[i,:])) - log N)
  upper = mean(T0) - mean(T1)

Sharding: y rows (i axis) split across 8 cores (64 rows each); x and MLP
params replicated. Each core computes its [64, 512] score block and emits
per-row partials (row lse, row sum of T1, diag element). Host combines.

Device layout: contraction dim k (=H, padded 400->512) on partitions.
The per-row score vector s[i, :] is routed to PSUM partition i via a
shifted one-hot stationary matrix: bsh[k] is [Pk, 128] with w2[k-chunk]
at column 64 and zeros elsewhere, so lhsT = bsh[k][:, 64-i : 128-i] has
w2 in column i.  All 256 matmuls accumulate into a single [64, 512]
PSUM bank (rows not owned by a matmul get exact +0).  The epilogue is
then fully batched: one Exp (with row-sum accum), one Ln (softplus, with
row-sum accum), and a masked diag reduce.

Per row i: 3 relu tiles on DVE (4x mode), 1 on ACT (Relu activation with
per-partition bias), 4 matmuls on PE (engine-bound at ~213 ns each).
"""

import numpy as np

N = 512          # number of samples
D = 400          # feature dim
H = 400          # hidden dim
NCORES = 8
NL = N // NCORES  # 64 y-rows per core
KP = 512          # padded contraction dim
KT = 4            # 128-partition k tiles
KSZ = [128, 128, 128, 16]   # real partitions per k tile (400 total)


def _build_program(b2val: float, enable_asserts: bool = False):
    import concourse.bacc as bacc
    import concourse.mybir as mybir
    import concourse.tile as tile

    fp16 = mybir.dt.float16
    f32 = mybir.dt.float32
    AF = mybir.ActivationFunctionType
    ALU = mybir.AluOpType

    nc = bacc.Bacc(
        "TRN2",
        target_bir_lowering=False,
        debug=False,
        enable_asserts=enable_asserts,
    )

    xT = nc.dram_tensor("xT", [KP, N], fp16, kind="ExternalInput")
    w1xT = nc.dram_tensor("w1xT", [KP, H], fp16, kind="ExternalInput")
    w1yT = nc.dram_tensor("w1yT", [KP, H], fp16, kind="ExternalInput")
    yT = nc.dram_tensor("yT", [KP, NL], fp16, kind="ExternalInput")
    b1c = nc.dram_tensor("b1c", [KP, 1], f32, kind="ExternalInput")
    bsh = nc.dram_tensor("bsh", [KP, 128], fp16, kind="ExternalInput")
    maskd = nc.dram_tensor("maskd", [NL, N], f32, kind="ExternalInput")

    out_o = nc.dram_tensor("out_o", [NL, 4], f32, kind="ExternalOutput")

    with tile.TileContext(nc) as tc:
        with (
            tc.tile_pool(name="const", bufs=1) as cpool,
            tc.tile_pool(name="work", bufs=24) as wpool,
            tc.tile_pool(name="ppro", bufs=4, space="PSUM") as ppro,
            tc.tile_pool(name="pmain", bufs=1, space="PSUM") as pmain,
        ):
            # ---- input DMAs, ordered by first use ----
            xt, w1x = [], []
            for k in range(KT):
                sl = slice(k * 128, (k + 1) * 128)
                t = cpool.tile([128, N], fp16, name=f"xt{k}")
                nc.sync.dma_start(out=t, in_=xT[sl, :])
                xt.append(t)
                t = cpool.tile([128, H], fp16, name=f"w1x{k}")
                nc.sync.dma_start(out=t, in_=w1xT[sl, :])
                w1x.append(t)
            w1y, yt = [], []
            for k in range(KT):
                sl = slice(k * 128, (k + 1) * 128)
                t = cpool.tile([128, H], fp16, name=f"w1y{k}")
                nc.sync.dma_start(out=t, in_=w1yT[sl, :])
                w1y.append(t)
                t = cpool.tile([128, NL], fp16, name=f"yt{k}")
                nc.sync.dma_start(out=t, in_=yT[sl, :])
                yt.append(t)
            b1l, bshl = [], []
            for k in range(KT):
                sl = slice(k * 128, k * 128 + KSZ[k])
                t = cpool.tile([KSZ[k], 1], f32, name=f"b1t{k}")
                nc.sync.dma_start(out=t, in_=b1c[sl, :])
                b1l.append(t)
                t = cpool.tile([KSZ[k], 128], fp16, name=f"bsh{k}")
                nc.sync.dma_start(out=t, in_=bsh[sl, :])
                bshl.append(t)
            mask = cpool.tile([NL, N], f32, name="mask")
            nc.sync.dma_start(out=mask, in_=maskd[:, :])
            b2t = cpool.tile([NL, 1], f32, name="b2t")
            nc.vector.memset(b2t, b2val)
            onet = cpool.tile([NL, 1], f32, name="onet")
            nc.vector.memset(onet, 1.0)
            n512t = cpool.tile([NL, 1], f32, name="n512t")
            nc.vector.memset(n512t, float(N))
            zt = cpool.tile([128, 1], f32, name="zt")
            nc.vector.memset(zt, 0.0)

            # ---- prologue: hx (fp16, m-tiled) and hyb = hy + b1 (f32) ----
            MSZ = KSZ  # m tiling of H matches k tiling
            hx, hyb = [], []
            for m in range(KT):
                msl = slice(m * 128, m * 128 + MSZ[m])
                ph = ppro.tile([MSZ[m], N], f32, name=f"ph{m}", tag="pp")
                for k in range(KT):
                    nc.tensor.matmul(
                        ph, lhsT=w1x[k][:, msl], rhs=xt[k],
                        start=(k == 0), stop=(k == KT - 1),
                    )
                hxm = cpool.tile([MSZ[m], N], fp16, name=f"hx{m}")
                if m % 2 == 0:
                    nc.vector.tensor_copy(out=hxm, in_=ph)
                else:
                    nc.scalar.activation(
                        out=hxm, in_=ph, func=AF.Copy, bias=0.0, scale=1.0,
                    )
                hx.append(hxm)
            for m in range(KT):
                msl = slice(m * 128, m * 128 + MSZ[m])
                py = ppro.tile([MSZ[m], NL], f32, name=f"py{m}", tag="pp")
                for k in range(KT):
                    nc.tensor.matmul(
                        py, lhsT=w1y[k][:, msl], rhs=yt[k],
                        start=(k == 0), stop=(k == KT - 1),
                    )
                hybm = cpool.tile([MSZ[m], NL], f32, name=f"hyb{m}")
                nc.vector.tensor_scalar_add(hybm, py, b1l[m])
                hyb.append(hybm)

            # ---- main loop: accumulate s rows into PS[i, :] ----
            ps = pmain.tile([NL, N], f32, name="ps", tag="ps")
            for i in range(NL):
                rlist = []
                for k in range(KT):
                    r = wpool.tile([KSZ[k], N], fp16, name=f"r{k}", tag=f"r{k}")
                    if k < 3:
                        nc.vector.tensor_scalar(
                            out=r, in0=hx[k],
                            scalar1=hyb[k][:, i : i + 1], scalar2=0.0,
                            op0=ALU.add, op1=ALU.max,
                        )
                    else:
                        nc.scalar.activation(
                            out=r, in_=hx[k], func=AF.Relu,
                            bias=hyb[k][:, i : i + 1], scale=1.0,
                        )
                    rlist.append(r)
                for k in range(KT):
                    nc.tensor.matmul(
                        ps, lhsT=bshl[k][:, 64 - i : 128 - i], rhs=rlist[k],
                        start=(i == 0 and k == 0), stop=(i == NL - 1 and k == KT - 1),
                    )

            # ---- batched epilogue ----
            e2 = cpool.tile([NL, N], f32, name="e2")
            t1 = cpool.tile([NL, N], f32, name="t1")
            tmp = cpool.tile([NL, N], f32, name="tmp")
            rr = cpool.tile([NL, 1], f32, name="rr")
            out3 = cpool.tile([NL, 4], f32, name="out3")
            nc.vector.memset(out3, 0.0)
            # E = exp(s + b2); rr[i] = sum_j E
            nc.scalar.activation(
                out=e2, in_=ps, func=AF.Exp, bias=b2t, scale=1.0,
                accum_out=rr,
            )
            # lse_i = log(N + sum_j e^s) -> col 0
            nc.scalar.activation(
                out=out3[:, 0:1], in_=rr, func=AF.Ln, bias=n512t, scale=1.0,
            )
            # T1 = log(1 + E) = softplus(s); rs row sums -> col 1
            nc.scalar.activation(
                out=t1, in_=e2, func=AF.Ln, bias=onet, scale=1.0,
                accum_out=out3[:, 1:2],
            )
            # t0 = diag(T1) -> col 2
            nc.vector.tensor_tensor(out=tmp, in0=t1, in1=mask, op=ALU.mult)
            nc.vector.reduce_sum(
                out=out3[:, 2:3], in_=tmp, axis=mybir.AxisListType.X
            )
            nc.sync.dma_start(out=out_o[:, :], in_=out3)

    nc.compile()
    return nc


def _make_in_maps(x, y, W1, b1, W2):
    f16 = np.float16
    xTp = np.zeros((KP, N), f16)
    xTp[:D, :] = x.T.astype(f16)
    w1xTp = np.zeros((KP, H), f16)
    w1xTp[:D, :] = W1[:, :D].T.astype(f16)
    w1yTp = np.zeros((KP, H), f16)
    w1yTp[:D, :] = W1[:, D:].T.astype(f16)
    b1p = np.zeros((KP, 1), np.float32)
    b1p[:H, 0] = b1
    bshp = np.zeros((KP, 128), f16)
    bshp[:H, 64] = W2[0].astype(f16)

    in_maps = []
    for c in range(NCORES):
        yTp = np.zeros((KP, NL), f16)
        yTp[:D, :] = y[c * NL : (c + 1) * NL, :].T.astype(f16)
        maskp = np.zeros((NL, N), np.float32)
        maskp[np.arange(NL), c * NL + np.arange(NL)] = 1.0
        in_maps.append(
            {
                "xT": xTp, "w1xT": w1xTp, "w1yT": w1yTp, "yT": yTp,
                "b1c": b1p, "bsh": bshp, "maskd": maskp,
            }
        )
    return in_maps


def _combine(results):
    lse_all = np.concatenate([r["out_o"][:, 0].astype(np.float64) for r in results])
    rs_all = np.concatenate([r["out_o"][:, 1].astype(np.float64) for r in results])
    t0_all = np.concatenate([r["out_o"][:, 2].astype(np.float64) for r in results])
    t0_mean = t0_all.mean()
    lower = t0_mean - (lse_all.mean() - np.log(np.float64(N)))
    upper = t0_mean - rs_all.mean() / N
    return np.float32(lower), np.float32(upper)


def kernel(x_samples, y_samples, W1, b1, W2, b2, _trace=False):
    from concourse.bass_utils import run_bass_kernel_spmd

    nc = _build_program(float(np.float32(b2[0])))
    in_maps = _make_in_maps(
        np.asarray(x_samples, np.float32),
        np.asarray(y_samples, np.float32),
        np.asarray(W1, np.float32),
        np.asarray(b1, np.float32),
        np.asarray(W2, np.float32),
    )
    res = run_bass_kernel_spmd(
        nc, in_maps, core_ids=list(range(NCORES)), trace=_trace
    )
    out = _combine(res.results)
    if _trace:
        return out, res
    return out


# revision 9
# speedup vs baseline: 1.2172x; 1.0866x over previous
"""CLUB-NCE loss kernel for 8x Trainium2 NeuronCores (Bass/Tile).

Math (reference):
  hx = x @ W1x.T, hy = y @ W1y.T            [N, H]
  s[i,j]  = W2 . relu(hy[i] + hx[j] + b1) + b2
  T1[i,j] = softplus(s[i,j]); T0[i] = T1[i,i]
  lower = mean(T0) - (mean_i(logsumexp_j(T1[i,:])) - log N)
  upper = mean(T0) - mean(T1)

Sharding: y rows (i axis) split across 8 cores (64 rows each); x and MLP
params replicated. Each core computes its [64, 512] score block and emits
per-row partials (row sum of e^s, row sum of T1, diag e^s). Host combines.

Device design notes:
 - contraction dim k (=H=400) on partitions, tiled [128,128,128,16(+1)].
 - score row i is routed to PSUM partition i via a shifted one-hot
   stationary matrix: bsh[k] is [Pk, 128] with w2[k-chunk] at column 64,
   so lhsT = bsh[k][:, 64-ii : 96-ii] puts w2 in column ii.  All matmuls
   of a 32-row half accumulate into one [32, 512] PSUM bank; rows not
   owned by a matmul get exact +0.
 - b1 is folded into the hy matmul as a virtual k=400 row (w1y row 400
   holds b1, y row 400 holds 1.0).
 - two 32-row halves so half A's epilogue (Exp + softplus row sums on
   ACT, masked diag on DVE) hides under half B's matmuls.
 - per row: relu tiles k=0..2 on DVE (4x mode), k=3 on ACT; 4 matmuls.
 - all inputs arrive in 6 DMAs (one [*,1504] fp16 slab per k-tile +
   fp16 diag mask) to minimize serialized HWDGE occupancy.
 - host finishes: lse_i = log(N + rr_i), t0_i = log(1 + ed_i), means.
"""

import numpy as np

N = 512          # number of samples
D = 400          # feature dim
H = 400          # hidden dim
NCORES = 8
NL = N // NCORES  # 64 y-rows per core
NH = NL // 2      # 32 rows per half
KP = 512          # padded contraction dim
KT = 4            # k tiles
KSZ = [128, 128, 128, 16]    # real k per tile (400 total)
KSZY = [128, 128, 128, 17]   # hy matmul k per tile (incl. bias row)
# consolidated input slab columns: x | w1x | w1y | yt | bsh
CX, CW1X, CW1Y, CYT, CBSH = 0, 512, 912, 1312, 1376
CTOT = 1504


def _build_program(b2val: float, enable_asserts: bool = False):
    import concourse.bacc as bacc
    import concourse.mybir as mybir
    import concourse.tile as tile

    fp16 = mybir.dt.float16
    f32 = mybir.dt.float32
    AF = mybir.ActivationFunctionType
    ALU = mybir.AluOpType

    nc = bacc.Bacc(
        "TRN2",
        target_bir_lowering=False,
        debug=False,
        enable_asserts=enable_asserts,
    )

    slab = nc.dram_tensor("slab", [401, CTOT], fp16, kind="ExternalInput")
    # diag mask, halves side by side: [:, 0:512] half A, [:, 512:1024] half B
    maskd = nc.dram_tensor("maskd", [NH, 2 * N], fp16, kind="ExternalInput")
    out_o = nc.dram_tensor("out_o", [NL, 4], f32, kind="ExternalOutput")

    with tile.TileContext(nc) as tc:
        with (
            tc.tile_pool(name="const", bufs=1) as cpool,
            tc.tile_pool(name="work", bufs=24) as wpool,
            tc.tile_pool(name="epi", bufs=2) as epool,
            tc.tile_pool(name="ppro", bufs=4, space="PSUM") as ppro,
            tc.tile_pool(name="pmain", bufs=2, space="PSUM") as pmain,
        ):
            # one table load (natural_log_exp_and_others: copy/relu/exp/ln)
            # hidden under the input DMAs instead of mid-epilogue
            nc.scalar.add_instruction(
                mybir.InstLoadActFuncSet(
                    name=nc.get_next_instruction_name(),
                    act_func_set_id=6,
                    engine=mybir.EngineType.Activation,
                    ins=[],
                    outs=[],
                )
            )

            # ---- input DMAs: one slab per k-tile + mask ----
            sl_t = []
            for k in range(KT):
                t = cpool.tile([KSZY[k], CTOT], fp16, name=f"slab{k}")
                nc.sync.dma_start(
                    out=t, in_=slab[k * 128 : k * 128 + KSZY[k], :]
                )
                sl_t.append(t)
            mask = cpool.tile([NH, 2 * N], fp16, name="mask")
            nc.sync.dma_start(out=mask, in_=maskd[:, :])

            xt = [sl_t[k][: KSZ[k], CX : CX + N] for k in range(KT)]
            w1x = [sl_t[k][: KSZ[k], CW1X : CW1X + H] for k in range(KT)]
            w1y = [sl_t[k][: KSZY[k], CW1Y : CW1Y + H] for k in range(KT)]
            yt = [sl_t[k][: KSZY[k], CYT : CYT + NL] for k in range(KT)]
            bshl = [sl_t[k][: KSZ[k], CBSH : CBSH + 128] for k in range(KT)]

            b2t = cpool.tile([NH, 1], f32, name="b2t")
            nc.vector.memset(b2t, b2val)
            onet = cpool.tile([NH, 1], f32, name="onet")
            nc.vector.memset(onet, 1.0)
            out3 = cpool.tile([NL, 4], f32, name="out3")
            nc.vector.memset(out3, 0.0)

            # ---- prologue: hx (fp16, m-tiled) and hyb = hy + b1 (f32) ----
            MSZ = KSZ
            hx, hyb = [], []
            for m in range(KT):
                msl = slice(m * 128, m * 128 + MSZ[m])
                ph = ppro.tile([MSZ[m], N], f32, name=f"ph{m}", tag="pp")
                for k in range(KT):
                    nc.tensor.matmul(
                        ph, lhsT=w1x[k][:, msl], rhs=xt[k],
                        start=(k == 0), stop=(k == KT - 1),
                    )
                hxm = cpool.tile([MSZ[m], N], fp16, name=f"hx{m}")
                if m % 2 == 0:
                    nc.vector.tensor_copy(out=hxm, in_=ph)
                else:
                    nc.scalar.activation(
                        out=hxm, in_=ph, func=AF.Copy, bias=0.0, scale=1.0,
                    )
                hx.append(hxm)
            for m in range(KT):
                msl = slice(m * 128, m * 128 + MSZ[m])
                py = ppro.tile([MSZ[m], NL], f32, name=f"py{m}", tag="pp")
                for k in range(KT):
                    nc.tensor.matmul(
                        py, lhsT=w1y[k][:, msl], rhs=yt[k],
                        start=(k == 0), stop=(k == KT - 1),
                    )
                hybm = cpool.tile([MSZ[m], NL], f32, name=f"hyb{m}")
                nc.vector.tensor_copy(out=hybm, in_=py)
                hyb.append(hybm)

            # ---- main loop: two 32-row halves ----
            def emit_row(half, ii):
                i = half * NH + ii
                rlist = []
                for k in range(KT):
                    r = wpool.tile(
                        [KSZ[k], N], fp16, name=f"r{k}", tag=f"r{k}"
                    )
                    if k < 3:
                        nc.vector.tensor_scalar(
                            out=r, in0=hx[k],
                            scalar1=hyb[k][:, i : i + 1], scalar2=0.0,
                            op0=ALU.add, op1=ALU.max,
                        )
                    else:
                        nc.scalar.activation(
                            out=r, in_=hx[k], func=AF.Relu,
                            bias=hyb[k][:, i : i + 1], scale=1.0,
                        )
                    rlist.append(r)
                for k in range(KT):
                    nc.tensor.matmul(
                        ps_h[half], lhsT=bshl[k][:, 64 - ii : 96 - ii],
                        rhs=rlist[k],
                        start=(ii == 0 and k == 0),
                        stop=(ii == NH - 1 and k == KT - 1),
                    )

            def emit_epilogue(half):
                osl = slice(half * NH, (half + 1) * NH)
                e2 = epool.tile([NH, N], fp16, name="e2", tag="e2")
                t1s = epool.tile([NH, N], fp16, name="t1s", tag="t1s")
                tmp = epool.tile([NH, N], fp16, name="tmp", tag="tmp")
                # E = exp(s + b2); rr = row sums of E
                nc.scalar.activation(
                    out=e2, in_=ps_h[half], func=AF.Exp, bias=b2t, scale=1.0,
                    accum_out=out3[osl, 0:1],
                )
                # T1 = log(1 + E); rs = row sums of T1
                nc.scalar.activation(
                    out=t1s, in_=e2, func=AF.Ln, bias=onet, scale=1.0,
                    accum_out=out3[osl, 1:2],
                )
                # ed = diag(E)
                nc.vector.tensor_tensor(
                    out=tmp, in0=e2, in1=mask[:, half * N : (half + 1) * N],
                    op=ALU.mult,
                )
                nc.vector.reduce_sum(
                    out=out3[osl, 2:3], in_=tmp, axis=mybir.AxisListType.X
                )

            ps_h = [
                pmain.tile([NH, N], f32, name=f"ps{h}", tag=f"ps{h}")
                for h in range(2)
            ]
            for ii in range(NH):
                emit_row(0, ii)
            for ii in range(NH):
                emit_row(1, ii)
                if ii == 3:
                    emit_epilogue(0)
            emit_epilogue(1)
            nc.sync.dma_start(out=out_o[:, :], in_=out3)

    nc.compile()
    return nc


def _make_in_maps(x, y, W1, b1, W2):
    f16 = np.float16
    slab = np.zeros((401, CTOT), f16)
    slab[:D, CX : CX + N] = x.T.astype(f16)
    slab[:D, CW1X : CW1X + H] = W1[:, :D].T.astype(f16)
    slab[:D, CW1Y : CW1Y + H] = W1[:, D:].T.astype(f16)
    slab[400, CW1Y : CW1Y + H] = b1.astype(f16)
    slab[:H, CBSH + 64] = W2[0].astype(f16)

    in_maps = []
    for c in range(NCORES):
        s = slab.copy()
        s[:D, CYT : CYT + NL] = y[c * NL : (c + 1) * NL, :].T.astype(f16)
        s[400, CYT : CYT + NL] = 1.0
        maskp = np.zeros((NH, 2 * N), f16)
        for h in range(2):
            rows = np.arange(NH)
            maskp[rows, h * N + c * NL + h * NH + rows] = 1.0
        in_maps.append({"slab": s, "maskd": maskp})
    return in_maps


def _combine(results):
    rr = np.concatenate([r["out_o"][:, 0].astype(np.float64) for r in results])
    rs = np.concatenate([r["out_o"][:, 1].astype(np.float64) for r in results])
    ed = np.concatenate([r["out_o"][:, 2].astype(np.float64) for r in results])
    lse = np.log(np.float64(N) + rr)
    t0 = np.log1p(ed)
    t0_mean = t0.mean()
    lower = t0_mean - (lse.mean() - np.log(np.float64(N)))
    upper = t0_mean - rs.mean() / N
    return np.float32(lower), np.float32(upper)


def kernel(x_samples, y_samples, W1, b1, W2, b2, _trace=False):
    from concourse.bass_utils import run_bass_kernel_spmd

    nc = _build_program(float(np.float32(b2[0])))
    in_maps = _make_in_maps(
        np.asarray(x_samples, np.float32),
        np.asarray(y_samples, np.float32),
        np.asarray(W1, np.float32),
        np.asarray(b1, np.float32),
        np.asarray(W2, np.float32),
    )
    res = run_bass_kernel_spmd(
        nc, in_maps, core_ids=list(range(NCORES)), trace=_trace
    )
    out = _combine(res.results)
    if _trace:
        return out, res
    return out


# revision 13
# speedup vs baseline: 1.4480x; 1.1896x over previous
"""CLUB-NCE loss kernel for 8x Trainium2 NeuronCores (Bass/Tile).

Math (reference):
  hx = x @ W1x.T, hy = y @ W1y.T            [N, H]
  s[i,j]  = W2 . relu(hy[i] + hx[j] + b1) + b2
  T1[i,j] = softplus(s[i,j]); T0[i] = T1[i,i]
  lower = mean(T0) - (mean_i(logsumexp_j(T1[i,:])) - log N)
  upper = mean(T0) - mean(T1)

Sharding: y rows (i axis) split across 8 cores (64 rows each); x and MLP
params replicated. Each core computes its [64, 512] score block and emits
per-row partials (row sum of e^s, row sum of T1, diag e^s). Host combines.

Device design notes:
 - contraction dim k (=H=400) on partitions, tiled [128,128,128,16(+1)].
 - score row i is routed to PSUM partition i via a shifted one-hot
   stationary matrix: bsh[k] is [Pk, 128] with w2[k-chunk] at column 64,
   so lhsT = bsh[k][:, 64-ii : 96-ii] puts w2 in column ii.  All matmuls
   of a 32-row half accumulate into one [32, 512] PSUM bank; rows not
   owned by a matmul get exact +0.
 - the 16-deep k3 tail tiles of 4 consecutive rows are packed into one
   [64, 512] rhs and contracted by a single matmul using a banded
   stationary B3 (B3[p, 28 + p//16] = w2[384 + p%16], window slid by
   4 columns per quad): 13 matmuls per 4 rows instead of 16.
 - b1 is folded into the hy matmul as a virtual k=400 row.
 - PE warmup: dummy matmuls keep the tensor engine continuously busy
   from t~0.7us so it reaches full clock before real work, and filler
   dummies bridge the prologue->main handoff (any idle gap drops the
   PE clock for ~3us).
 - prologue is k-batch ordered (one batch per arriving input slab) so
   the PE never stalls mid-prologue.
 - two 32-row halves; half A's epilogue (Exp + softplus row sums on ACT,
   masked diag on DVE) hides under half B's matmuls.
 - per quad of rows: 12+1 relu tiles split DVE (4x mode) / ACT.
 - inputs arrive in 6 DMAs (one [*,1504] fp16 slab per k-tile + banded
   B3 + fp16 diag mask) to minimize serialized HWDGE occupancy.
 - host finishes: lse_i = log(N + rr_i), t0_i = log(1 + ed_i), means.
"""

import numpy as np

N = 512          # number of samples
D = 400          # feature dim
H = 400          # hidden dim
NCORES = 8
NL = N // NCORES  # 64 y-rows per core
NH = NL // 2      # 32 rows per half
KT = 4            # k tiles
KSZ = [128, 128, 128, 16]    # real k per tile (400 total)
KSZY = [128, 128, 128, 17]   # hy matmul k per tile (incl. bias row)
# consolidated input slab columns: x | w1x(+m3 dup) | w1y(+m3 dup) | yt | bsh
CX, CW1X, CW1Y, CYT, CBSH = 0, 512, 928, 1344, 1408
CTOT = 1536
MSZ = [128, 128, 128, 32]    # H-tile partition sizes (m3 duplicated 16+16)
NWARM = 16        # PE warmup dummy matmuls
NFILL = 10        # PE filler dummies between prologue and main loop
DROWS = 384       # dummy matmul free size


def _build_program(b2val: float, enable_asserts: bool = False):
    import concourse.bacc as bacc
    import concourse.mybir as mybir
    import concourse.tile as tile

    fp16 = mybir.dt.float16
    f32 = mybir.dt.float32
    AF = mybir.ActivationFunctionType
    ALU = mybir.AluOpType

    nc = bacc.Bacc(
        "TRN2",
        target_bir_lowering=False,
        debug=False,
        enable_asserts=enable_asserts,
    )

    slab = nc.dram_tensor("slab", [401, CTOT], fp16, kind="ExternalInput")
    b3d = nc.dram_tensor("b3d", [128, 60], fp16, kind="ExternalInput")
    # diag mask, halves side by side: [:, 0:512] half A, [:, 512:1024] half B
    maskd = nc.dram_tensor("maskd", [NH, 2 * N], fp16, kind="ExternalInput")
    out_o = nc.dram_tensor("out_o", [NL, 4], f32, kind="ExternalOutput")

    with tile.TileContext(nc) as tc:
        with (
            tc.tile_pool(name="const", bufs=1) as cpool,
            tc.tile_pool(name="work", bufs=24) as wpool,
            tc.tile_pool(name="rq", bufs=4) as rqpool,
            tc.tile_pool(name="epi", bufs=2) as epool,
            tc.tile_pool(name="ppro", bufs=4, space="PSUM") as ppro,
            tc.tile_pool(name="phy", bufs=1, space="PSUM") as phy,
            tc.tile_pool(name="pmain", bufs=1, space="PSUM") as pmain,
            tc.tile_pool(name="pdum", bufs=1, space="PSUM") as pdum,
        ):
            # one table load (natural_log_exp_and_others: copy/relu/exp/ln)
            # hidden under the input DMAs instead of mid-epilogue
            nc.scalar.add_instruction(
                mybir.InstLoadActFuncSet(
                    name=nc.get_next_instruction_name(),
                    act_func_set_id=6,
                    engine=mybir.EngineType.Activation,
                    ins=[],
                    outs=[],
                )
            )

            # ---- input DMAs: one slab per k-tile + B3 + mask ----
            sl_t = []
            for k in range(KT):
                t = cpool.tile([KSZY[k], CTOT], fp16, name=f"slab{k}")
                nc.sync.dma_start(
                    out=t, in_=slab[k * 128 : k * 128 + KSZY[k], :]
                )
                sl_t.append(t)
            b3 = cpool.tile([128, 60], fp16, name="b3")
            nc.sync.dma_start(out=b3, in_=b3d[:, :])
            mask = cpool.tile([NH, 2 * N], fp16, name="mask")
            nc.sync.dma_start(out=mask, in_=maskd[:, :])

            xt = [sl_t[k][: KSZ[k], CX : CX + N] for k in range(KT)]
            w1x = [sl_t[k][: KSZ[k], CW1X : CW1X + H + 16] for k in range(KT)]
            w1y = [sl_t[k][: KSZY[k], CW1Y : CW1Y + H + 16] for k in range(KT)]
            yt = [sl_t[k][: KSZY[k], CYT : CYT + NL] for k in range(KT)]
            bshl = [sl_t[k][: KSZ[k], CBSH : CBSH + 128] for k in range(KT)]

            b2t = cpool.tile([NH, 1], f32, name="b2t")
            nc.vector.memset(b2t, b2val)
            onet = cpool.tile([NH, 1], f32, name="onet")
            nc.vector.memset(onet, 1.0)
            out3 = cpool.tile([NL, 4], f32, name="out3")
            nc.vector.memset(out3, 0.0)

            # ---- PE warmup: keep the tensor engine busy from t~0 ----
            dumw = cpool.tile([128, 1], fp16, name="dumw")
            nc.vector.memset(dumw, 0.0)
            dumr = cpool.tile([128, DROWS], fp16, name="dumr")
            nc.vector.memset(dumr, 0.0)
            pd = pdum.tile([1, DROWS], f32, name="pd", tag="pd")

            def dummies(n):
                for _ in range(n):
                    nc.tensor.matmul(pd, lhsT=dumw, rhs=dumr,
                                     start=True, stop=True)

            dummies(NWARM)

            # ---- prologue, k-batch ordered: hy then hx per arriving slab ----
            pyall = phy.tile([128, 4 * NL], f32, name="pyall", tag="py")
            ph = [
                ppro.tile([MSZ[m], N], f32, name=f"ph{m}", tag="pp")
                for m in range(KT)
            ]
            # hx matmuls k-batched (one batch per arriving slab, no stalls;
            # the 4 ph banks are distinct so group interleaving is safe)
            for k in range(KT):
                for m in range(KT):
                    msl = slice(m * 128, m * 128 + MSZ[m])
                    nc.tensor.matmul(
                        ph[m], lhsT=w1x[k][:, msl], rhs=xt[k],
                        start=(k == 0), stop=(k == KT - 1),
                    )
            # hx tiles to fp16 (DVE/ACT split, overlaps the hy matmuls)
            hx = []
            for m in range(KT):
                hxm = cpool.tile([MSZ[m], N], fp16, name=f"hx{m}")
                if m % 2 == 0:
                    nc.vector.tensor_copy(out=hxm, in_=ph[m])
                else:
                    nc.scalar.activation(
                        out=hxm, in_=ph[m], func=AF.Copy, bias=0.0, scale=1.0,
                    )
                hx.append(hxm)
            # hy blocks share one PSUM bank: groups must be sequential per
            # block (same-bank interleaved start/stop corrupts accumulation)
            for m in range(KT):
                msl = slice(m * 128, m * 128 + MSZ[m])
                for k in range(KT):
                    nc.tensor.matmul(
                        pyall[: MSZ[m], m * NL : (m + 1) * NL],
                        lhsT=w1y[k][:, msl], rhs=yt[k],
                        start=(k == 0), stop=(k == KT - 1),
                    )
            # hyb = hy + b1 (one f32 copy)
            hyball = cpool.tile([128, 4 * NL], f32, name="hyball")
            nc.vector.tensor_copy(out=hyball, in_=pyall)

            def hyb(m, i):  # per-partition scalar for H-tile m, row i
                return hyball[: MSZ[m], m * NL + i : m * NL + i + 1]

            dummies(NFILL)  # bridge prologue->main while copies drain

            # ---- main loop: two 32-row halves, quads of 4 rows ----
            def emit_quad(half, q):
                rq = rqpool.tile([128, N], fp16, name="rq", tag="rq")
                rks = []
                for a in range(4):
                    i = half * NH + 4 * q + a
                    for k in range(3):
                        r = wpool.tile([128, N], fp16, name=f"r{k}",
                                       tag=f"r{k}")
                        nc.vector.tensor_scalar(
                            out=r, in0=hx[k], scalar1=hyb(k, i), scalar2=0.0,
                            op0=ALU.add, op1=ALU.max,
                        )
                        rks.append(r)
                    if a == 0:
                        nc.vector.tensor_scalar(
                            out=rq[0:32, :], in0=hx[3],
                            scalar1=hyb(3, i), scalar2=0.0,
                            op0=ALU.add, op1=ALU.max,
                        )
                    else:
                        nc.scalar.activation(
                            out=rq[32 * a : 32 * (a + 1), :], in_=hx[3],
                            func=AF.Relu, bias=hyb(3, i), scale=1.0,
                        )
                for a in range(4):
                    ii = 4 * q + a
                    for k in range(3):
                        nc.tensor.matmul(
                            ps_h[half], lhsT=bshl[k][:, 64 - ii : 96 - ii],
                            rhs=rks[3 * a + k],
                            start=(q == 0 and a == 0 and k == 0), stop=False,
                        )
                nc.tensor.matmul(
                    ps_h[half], lhsT=b3[:, 28 - 4 * q : 60 - 4 * q], rhs=rq,
                    start=False, stop=(q == NH // 4 - 1),
                )

            def emit_epilogue(half):
                osl = slice(half * NH, (half + 1) * NH)
                e2 = epool.tile([NH, N], fp16, name="e2", tag="e2")
                t1s = epool.tile([NH, N], fp16, name="t1s", tag="t1s")
                tmp = epool.tile([NH, N], fp16, name="tmp", tag="tmp")
                # E = exp(s + b2); rr = row sums of E
                nc.scalar.activation(
                    out=e2, in_=ps_h[half], func=AF.Exp, bias=b2t, scale=1.0,
                    accum_out=out3[osl, 0:1],
                )
                # T1 = log(1 + E); rs = row sums of T1
                nc.scalar.activation(
                    out=t1s, in_=e2, func=AF.Ln, bias=onet, scale=1.0,
                    accum_out=out3[osl, 1:2],
                )
                # ed = diag(E)
                nc.vector.tensor_tensor(
                    out=tmp, in0=e2, in1=mask[:, half * N : (half + 1) * N],
                    op=ALU.mult,
                )
                nc.vector.reduce_sum(
                    out=out3[osl, 2:3], in_=tmp, axis=mybir.AxisListType.X
                )
                nc.sync.dma_start(out=out_o[osl, :], in_=out3[osl, :])

            ps_h = [
                pmain.tile([NH, N], f32, name=f"ps{h}", tag=f"ps{h}")
                for h in range(2)
            ]
            for q in range(NH // 4):
                emit_quad(0, q)
            for q in range(NH // 4):
                emit_quad(1, q)
                if q == 0:
                    emit_epilogue(0)
            emit_epilogue(1)

    nc.compile()
    return nc


def _make_in_maps(x, y, W1, b1, W2):
    f16 = np.float16
    slab = np.zeros((401, CTOT), f16)
    slab[:D, CX : CX + N] = x.T.astype(f16)
    w1xT = W1[:, :D].T.astype(f16)       # [D(k), H(m)]
    w1yT = W1[:, D:].T.astype(f16)
    slab[:D, CW1X : CW1X + H] = w1xT
    slab[:D, CW1X + H : CW1X + H + 16] = w1xT[:, 384:400]   # m3 dup
    slab[:D, CW1Y : CW1Y + H] = w1yT
    slab[:D, CW1Y + H : CW1Y + H + 16] = w1yT[:, 384:400]
    slab[400, CW1Y : CW1Y + H] = b1.astype(f16)
    slab[400, CW1Y + H : CW1Y + H + 16] = b1[384:400].astype(f16)
    slab[:H, CBSH + 64] = W2[0].astype(f16)
    b3p = np.zeros((128, 60), f16)
    p = np.arange(128)
    val = W2[0, 384 + (p % 32) % 16].astype(f16)
    b3p[p, 28 + p // 32] = np.where(p % 32 < 16, val, 0.0)

    in_maps = []
    for c in range(NCORES):
        s = slab.copy()
        s[:D, CYT : CYT + NL] = y[c * NL : (c + 1) * NL, :].T.astype(f16)
        s[400, CYT : CYT + NL] = 1.0
        maskp = np.zeros((NH, 2 * N), f16)
        rows = np.arange(NH)
        for h in range(2):
            maskp[rows, h * N + c * NL + h * NH + rows] = 1.0
        in_maps.append({"slab": s, "b3d": b3p, "maskd": maskp})
    return in_maps


def _combine(results):
    rr = np.concatenate([r["out_o"][:, 0].astype(np.float64) for r in results])
    rs = np.concatenate([r["out_o"][:, 1].astype(np.float64) for r in results])
    ed = np.concatenate([r["out_o"][:, 2].astype(np.float64) for r in results])
    lse = np.log(np.float64(N) + rr)
    t0 = np.log1p(ed)
    t0_mean = t0.mean()
    lower = t0_mean - (lse.mean() - np.log(np.float64(N)))
    upper = t0_mean - rs.mean() / N
    return np.float32(lower), np.float32(upper)


def kernel(x_samples, y_samples, W1, b1, W2, b2, _trace=False):
    from concourse.bass_utils import run_bass_kernel_spmd

    nc = _build_program(float(np.float32(b2[0])))
    in_maps = _make_in_maps(
        np.asarray(x_samples, np.float32),
        np.asarray(y_samples, np.float32),
        np.asarray(W1, np.float32),
        np.asarray(b1, np.float32),
        np.asarray(W2, np.float32),
    )
    res = run_bass_kernel_spmd(
        nc, in_maps, core_ids=list(range(NCORES)), trace=_trace
    )
    out = _combine(res.results)
    if _trace:
        return out, res
    return out


# revision 15
# speedup vs baseline: 1.5219x; 1.0510x over previous
"""CLUB-NCE loss kernel for 8x Trainium2 NeuronCores (Bass/Tile).

Math (reference):
  hx = x @ W1x.T, hy = y @ W1y.T            [N, H]
  s[i,j]  = W2 . relu(hy[i] + hx[j] + b1) + b2
  T1[i,j] = softplus(s[i,j]); T0[i] = T1[i,i]
  lower = mean(T0) - (mean_i(logsumexp_j(T1[i,:])) - log N)
  upper = mean(T0) - mean(T1)

Sharding: y rows (i axis) split across 8 cores (64 rows each); x and MLP
params replicated. Each core computes its [64, 512] score block and emits
per-row partials (row sum of e^s, row sum of T1, diag e^s). Host combines.

Device design notes:
 - contraction dim k (=H=400) on partitions, tiled [128,128,128,16(+1)].
 - score row i is routed to PSUM partition i via a shifted one-hot
   stationary matrix: bsh[k] is [Pk, 128] with w2[k-chunk] at column 64,
   so lhsT = bsh[k][:, 64-ii : 96-ii] puts w2 in column ii.  All matmuls
   of a 32-row half accumulate into one [32, 512] PSUM bank; rows not
   owned by a matmul get exact +0.
 - the 16-deep k3 tail tiles of 4 consecutive rows are packed into one
   [64, 512] rhs and contracted by a single matmul using a banded
   stationary B3 (B3[p, 28 + p//16] = w2[384 + p%16], window slid by
   4 columns per quad): 13 matmuls per 4 rows instead of 16.
 - b1 is folded into the hy matmul as a virtual k=400 row.
 - PE warmup: dummy matmuls keep the tensor engine continuously busy
   from t~0.7us so it reaches full clock before real work, and filler
   dummies bridge the prologue->main handoff (any idle gap drops the
   PE clock for ~3us).
 - prologue is k-batch ordered (one batch per arriving input slab) so
   the PE never stalls mid-prologue.
 - two 32-row halves; half A's epilogue (Exp + softplus row sums on ACT,
   masked diag on DVE) hides under half B's matmuls.
 - per quad of rows: 12+1 relu tiles split DVE (4x mode) / ACT.
 - inputs arrive in 6 DMAs (one [*,1504] fp16 slab per k-tile + banded
   B3 + fp16 diag mask) to minimize serialized HWDGE occupancy.
 - host finishes: lse_i = log(N + rr_i), t0_i = log(1 + ed_i), means.
"""

import numpy as np

N = 512          # number of samples
D = 400          # feature dim
H = 400          # hidden dim
NCORES = 8
NL = N // NCORES  # 64 y-rows per core
NH = NL // 2      # 32 rows per half
KT = 4            # k tiles
KSZ = [128, 128, 128, 16]    # real k per tile (400 total)
KSZY = [128, 128, 128, 17]   # hy matmul k per tile (incl. bias row)
# consolidated input slab columns: x | w1x(+m3 dup) | w1y(+m3 dup) | yt | bsh
CX, CW1X, CW1Y, CYT, CBSH = 0, 512, 928, 1344, 1408
CTOT = 1536
MSZ = [128, 128, 128, 32]    # H-tile partition sizes (m3 duplicated 16+16)
NWARM = 1         # PE warmup dummy matmuls (anchors the p-state ramp)
NFILL = 0         # PE filler dummies between prologue and main loop
DROWS = 384       # dummy matmul free size


def _build_program(b2val: float, enable_asserts: bool = False):
    import concourse.bacc as bacc
    import concourse.mybir as mybir
    import concourse.tile as tile

    fp16 = mybir.dt.float16
    f32 = mybir.dt.float32
    AF = mybir.ActivationFunctionType
    ALU = mybir.AluOpType

    nc = bacc.Bacc(
        "TRN2",
        target_bir_lowering=False,
        debug=False,
        enable_asserts=enable_asserts,
    )

    slab = nc.dram_tensor("slab", [401, CTOT], fp16, kind="ExternalInput")
    b3d = nc.dram_tensor("b3d", [128, 60], fp16, kind="ExternalInput")
    # diag mask, halves side by side: [:, 0:512] half A, [:, 512:1024] half B
    maskd = nc.dram_tensor("maskd", [NH, 2 * N], fp16, kind="ExternalInput")
    out_o = nc.dram_tensor("out_o", [NL, 4], f32, kind="ExternalOutput")

    with tile.TileContext(nc) as tc:
        with (
            tc.tile_pool(name="const", bufs=1) as cpool,
            tc.tile_pool(name="work", bufs=24) as wpool,
            tc.tile_pool(name="rq", bufs=4) as rqpool,
            tc.tile_pool(name="epi", bufs=2) as epool,
            tc.tile_pool(name="ppro", bufs=4, space="PSUM") as ppro,
            tc.tile_pool(name="phy", bufs=1, space="PSUM") as phy,
            tc.tile_pool(name="pmain", bufs=1, space="PSUM") as pmain,
            tc.tile_pool(name="pdum", bufs=1, space="PSUM") as pdum,
        ):
            # one table load (natural_log_exp_and_others: copy/relu/exp/ln)
            # hidden under the input DMAs instead of mid-epilogue
            nc.scalar.add_instruction(
                mybir.InstLoadActFuncSet(
                    name=nc.get_next_instruction_name(),
                    act_func_set_id=6,
                    engine=mybir.EngineType.Activation,
                    ins=[],
                    outs=[],
                )
            )

            # ---- input DMAs: one slab per k-tile + B3 + mask ----
            sl_t = []
            for k in range(KT):
                t = cpool.tile([KSZY[k], CTOT], fp16, name=f"slab{k}")
                nc.sync.dma_start(
                    out=t, in_=slab[k * 128 : k * 128 + KSZY[k], :]
                )
                sl_t.append(t)
            b3 = cpool.tile([128, 60], fp16, name="b3")
            nc.sync.dma_start(out=b3, in_=b3d[:, :])
            mask = cpool.tile([NH, 2 * N], fp16, name="mask")
            nc.sync.dma_start(out=mask, in_=maskd[:, :])

            xt = [sl_t[k][: KSZ[k], CX : CX + N] for k in range(KT)]
            w1x = [sl_t[k][: KSZ[k], CW1X : CW1X + H + 16] for k in range(KT)]
            w1y = [sl_t[k][: KSZY[k], CW1Y : CW1Y + H + 16] for k in range(KT)]
            yt = [sl_t[k][: KSZY[k], CYT : CYT + NL] for k in range(KT)]
            bshl = [sl_t[k][: KSZ[k], CBSH : CBSH + 128] for k in range(KT)]

            # ---- PE warmup: keep the tensor engine busy from t~0 ----
            dumw = cpool.tile([128, 1], fp16, name="dumw")
            nc.vector.memset(dumw, 0.0)
            dumr = cpool.tile([128, DROWS], fp16, name="dumr")
            nc.vector.memset(dumr, 0.0)
            pd = pdum.tile([1, DROWS], f32, name="pd", tag="pd")

            b2t = cpool.tile([NH, 1], f32, name="b2t")
            nc.vector.memset(b2t, b2val)
            onet = cpool.tile([NH, 1], f32, name="onet")
            nc.vector.memset(onet, 1.0)
            out3 = cpool.tile([NL, 4], f32, name="out3")
            nc.vector.memset(out3, 0.0)

            def dummies(n):
                for _ in range(n):
                    nc.tensor.matmul(pd, lhsT=dumw, rhs=dumr,
                                     start=True, stop=True)

            dummies(NWARM)

            # ---- prologue, k-batch ordered: hy then hx per arriving slab ----
            pyall = phy.tile([128, 4 * NL], f32, name="pyall", tag="py")
            ph = [
                ppro.tile([MSZ[m], N], f32, name=f"ph{m}", tag="pp")
                for m in range(KT)
            ]
            # hx matmuls k-batched (one batch per arriving slab, no stalls;
            # the 4 ph banks are distinct so group interleaving is safe)
            for k in range(KT):
                for m in range(KT):
                    msl = slice(m * 128, m * 128 + MSZ[m])
                    nc.tensor.matmul(
                        ph[m], lhsT=w1x[k][:, msl], rhs=xt[k],
                        start=(k == 0), stop=(k == KT - 1),
                    )
            # hx tiles to fp16 (DVE/ACT split, overlaps the hy matmuls)
            hx = []
            for m in range(KT):
                hxm = cpool.tile([MSZ[m], N], fp16, name=f"hx{m}")
                if m % 2 == 0:
                    nc.vector.tensor_copy(out=hxm, in_=ph[m])
                else:
                    nc.scalar.activation(
                        out=hxm, in_=ph[m], func=AF.Copy, bias=0.0, scale=1.0,
                    )
                hx.append(hxm)
            # hy blocks share one PSUM bank: groups must be sequential per
            # block (same-bank interleaved start/stop corrupts accumulation)
            for m in range(KT):
                msl = slice(m * 128, m * 128 + MSZ[m])
                for k in range(KT):
                    nc.tensor.matmul(
                        pyall[: MSZ[m], m * NL : (m + 1) * NL],
                        lhsT=w1y[k][:, msl], rhs=yt[k],
                        start=(k == 0), stop=(k == KT - 1),
                    )
            # hyb = hy + b1 (one f32 copy)
            hyball = cpool.tile([128, 4 * NL], f32, name="hyball")
            nc.vector.tensor_copy(out=hyball, in_=pyall)

            def hyb(m, i):  # per-partition scalar for H-tile m, row i
                return hyball[: MSZ[m], m * NL + i : m * NL + i + 1]

            dummies(NFILL)  # bridge prologue->main while copies drain

            # ---- main loop: two 32-row halves, quads of 4 rows ----
            def emit_quad(half, q):
                rq = rqpool.tile([128, N], fp16, name="rq", tag="rq")
                rks = []
                for a in range(4):
                    i = half * NH + 4 * q + a
                    for k in range(3):
                        r = wpool.tile([128, N], fp16, name=f"r{k}",
                                       tag=f"r{k}")
                        nc.vector.tensor_scalar(
                            out=r, in0=hx[k], scalar1=hyb(k, i), scalar2=0.0,
                            op0=ALU.add, op1=ALU.max,
                        )
                        rks.append(r)
                    if a == 0:
                        nc.vector.tensor_scalar(
                            out=rq[0:32, :], in0=hx[3],
                            scalar1=hyb(3, i), scalar2=0.0,
                            op0=ALU.add, op1=ALU.max,
                        )
                    else:
                        nc.scalar.activation(
                            out=rq[32 * a : 32 * (a + 1), :], in_=hx[3],
                            func=AF.Relu, bias=hyb(3, i), scale=1.0,
                        )
                for a in range(4):
                    ii = 4 * q + a
                    for k in range(3):
                        nc.tensor.matmul(
                            ps_h[half], lhsT=bshl[k][:, 64 - ii : 96 - ii],
                            rhs=rks[3 * a + k],
                            start=(q == 0 and a == 0 and k == 0), stop=False,
                        )
                nc.tensor.matmul(
                    ps_h[half], lhsT=b3[:, 28 - 4 * q : 60 - 4 * q], rhs=rq,
                    start=False, stop=(q == NH // 4 - 1),
                )

            def emit_epilogue(half):
                osl = slice(half * NH, (half + 1) * NH)
                e2 = epool.tile([NH, N], fp16, name="e2", tag="e2")
                t1s = epool.tile([NH, N], fp16, name="t1s", tag="t1s")
                tmp = epool.tile([NH, N], fp16, name="tmp", tag="tmp")
                # E = exp(s + b2); rr = row sums of E
                nc.scalar.activation(
                    out=e2, in_=ps_h[half], func=AF.Exp, bias=b2t, scale=1.0,
                    accum_out=out3[osl, 0:1],
                )
                # T1 = log(1 + E); rs = row sums of T1
                nc.scalar.activation(
                    out=t1s, in_=e2, func=AF.Ln, bias=onet, scale=1.0,
                    accum_out=out3[osl, 1:2],
                )
                # ed = diag(E)
                nc.vector.tensor_tensor(
                    out=tmp, in0=e2, in1=mask[:, half * N : (half + 1) * N],
                    op=ALU.mult,
                )
                nc.vector.reduce_sum(
                    out=out3[osl, 2:3], in_=tmp, axis=mybir.AxisListType.X
                )
                nc.sync.dma_start(out=out_o[osl, :], in_=out3[osl, :])

            ps_h = [
                pmain.tile([NH, N], f32, name=f"ps{h}", tag=f"ps{h}")
                for h in range(2)
            ]
            for q in range(NH // 4):
                emit_quad(0, q)
            for q in range(NH // 4):
                emit_quad(1, q)
                if q == 0:
                    emit_epilogue(0)
            emit_epilogue(1)

    nc.compile()
    return nc


def _make_in_maps(x, y, W1, b1, W2):
    f16 = np.float16
    slab = np.zeros((401, CTOT), f16)
    slab[:D, CX : CX + N] = x.T.astype(f16)
    w1xT = W1[:, :D].T.astype(f16)       # [D(k), H(m)]
    w1yT = W1[:, D:].T.astype(f16)
    slab[:D, CW1X : CW1X + H] = w1xT
    slab[:D, CW1X + H : CW1X + H + 16] = w1xT[:, 384:400]   # m3 dup
    slab[:D, CW1Y : CW1Y + H] = w1yT
    slab[:D, CW1Y + H : CW1Y + H + 16] = w1yT[:, 384:400]
    slab[400, CW1Y : CW1Y + H] = b1.astype(f16)
    slab[400, CW1Y + H : CW1Y + H + 16] = b1[384:400].astype(f16)
    slab[:H, CBSH + 64] = W2[0].astype(f16)
    b3p = np.zeros((128, 60), f16)
    p = np.arange(128)
    val = W2[0, 384 + (p % 32) % 16].astype(f16)
    b3p[p, 28 + p // 32] = np.where(p % 32 < 16, val, 0.0)

    in_maps = []
    for c in range(NCORES):
        s = slab.copy()
        s[:D, CYT : CYT + NL] = y[c * NL : (c + 1) * NL, :].T.astype(f16)
        s[400, CYT : CYT + NL] = 1.0
        maskp = np.zeros((NH, 2 * N), f16)
        rows = np.arange(NH)
        for h in range(2):
            maskp[rows, h * N + c * NL + h * NH + rows] = 1.0
        in_maps.append({"slab": s, "b3d": b3p, "maskd": maskp})
    return in_maps


def _combine(results):
    rr = np.concatenate([r["out_o"][:, 0].astype(np.float64) for r in results])
    rs = np.concatenate([r["out_o"][:, 1].astype(np.float64) for r in results])
    ed = np.concatenate([r["out_o"][:, 2].astype(np.float64) for r in results])
    lse = np.log(np.float64(N) + rr)
    t0 = np.log1p(ed)
    t0_mean = t0.mean()
    lower = t0_mean - (lse.mean() - np.log(np.float64(N)))
    upper = t0_mean - rs.mean() / N
    return np.float32(lower), np.float32(upper)


def kernel(x_samples, y_samples, W1, b1, W2, b2, _trace=False):
    from concourse.bass_utils import run_bass_kernel_spmd

    nc = _build_program(float(np.float32(b2[0])))
    in_maps = _make_in_maps(
        np.asarray(x_samples, np.float32),
        np.asarray(y_samples, np.float32),
        np.asarray(W1, np.float32),
        np.asarray(b1, np.float32),
        np.asarray(W2, np.float32),
    )
    res = run_bass_kernel_spmd(
        nc, in_maps, core_ids=list(range(NCORES)), trace=_trace
    )
    out = _combine(res.results)
    if _trace:
        return out, res
    return out


# revision 16
# speedup vs baseline: 1.5304x; 1.0056x over previous
"""CLUB-NCE loss kernel for 8x Trainium2 NeuronCores (Bass/Tile).

Math (reference):
  hx = x @ W1x.T, hy = y @ W1y.T            [N, H]
  s[i,j]  = W2 . relu(hy[i] + hx[j] + b1) + b2
  T1[i,j] = softplus(s[i,j]); T0[i] = T1[i,i]
  lower = mean(T0) - (mean_i(logsumexp_j(T1[i,:])) - log N)
  upper = mean(T0) - mean(T1)

Sharding: y rows (i axis) split across 8 cores (64 rows each); x and MLP
params replicated. Each core computes its [64, 512] score block and emits
per-row partials (row sum of e^s, row sum of T1, diag e^s). Host combines.

Device design notes:
 - contraction dim k (=H=400) on partitions, tiled [128,128,128,16(+1)].
 - score row i is routed to PSUM partition i via a shifted one-hot
   stationary matrix: bsh[k] is [Pk, 128] with w2[k-chunk] at column 64,
   so lhsT = bsh[k][:, 64-ii : 96-ii] puts w2 in column ii.  All matmuls
   of a 32-row half accumulate into one [32, 512] PSUM bank; rows not
   owned by a matmul get exact +0.
 - the 16-deep k3 tail tiles of 4 consecutive rows are packed into one
   [64, 512] rhs and contracted by a single matmul using a banded
   stationary B3 (B3[p, 28 + p//16] = w2[384 + p%16], window slid by
   4 columns per quad): 13 matmuls per 4 rows instead of 16.
 - b1 is folded into the hy matmul as a virtual k=400 row.
 - PE warmup: dummy matmuls keep the tensor engine continuously busy
   from t~0.7us so it reaches full clock before real work, and filler
   dummies bridge the prologue->main handoff (any idle gap drops the
   PE clock for ~3us).
 - prologue is k-batch ordered (one batch per arriving input slab) so
   the PE never stalls mid-prologue.
 - two 32-row halves; half A's epilogue (Exp + softplus row sums on ACT,
   masked diag on DVE) hides under half B's matmuls.
 - per quad of rows: 12+1 relu tiles split DVE (4x mode) / ACT.
 - inputs arrive in 6 DMAs (one [*,1504] fp16 slab per k-tile + banded
   B3 + fp16 diag mask) to minimize serialized HWDGE occupancy.
 - host finishes: lse_i = log(N + rr_i), t0_i = log(1 + ed_i), means.
"""

import numpy as np

N = 512          # number of samples
D = 400          # feature dim
H = 400          # hidden dim
NCORES = 8
NL = N // NCORES  # 64 y-rows per core
NH = NL // 2      # 32 rows per half
KT = 4            # k tiles
KSZ = [128, 128, 128, 16]    # real k per tile (400 total)
KSZY = [128, 128, 128, 17]   # hy matmul k per tile (incl. bias row)
# consolidated input slab columns: x | w1x(+m3 dup) | w1y(+m3 dup) | yt | bsh
CX, CW1X, CW1Y, CYT, CBSH = 0, 512, 928, 1344, 1408
CTOT = 1536
MSZ = [128, 128, 128, 32]    # H-tile partition sizes (m3 duplicated 16+16)
NWARM = 1         # PE warmup dummy matmuls (anchors the p-state ramp)
NFILL = 0         # PE filler dummies between prologue and main loop
DROWS = 384       # dummy matmul free size


def _build_program(b2val: float, enable_asserts: bool = False):
    import concourse.bacc as bacc
    import concourse.mybir as mybir
    import concourse.tile as tile

    fp16 = mybir.dt.float16
    f32 = mybir.dt.float32
    AF = mybir.ActivationFunctionType
    ALU = mybir.AluOpType

    nc = bacc.Bacc(
        "TRN2",
        target_bir_lowering=False,
        debug=False,
        enable_asserts=enable_asserts,
    )

    slab = nc.dram_tensor("slab", [401, CTOT], fp16, kind="ExternalInput")
    b3d = nc.dram_tensor("b3d", [128, 60], fp16, kind="ExternalInput")
    # x columns are rotated per core so the diag block sits at columns
    # [h*32, h*32+32) of half h; the mask is just a [32,32] identity
    maskd = nc.dram_tensor("maskd", [NH, NH], fp16, kind="ExternalInput")
    out_o = nc.dram_tensor("out_o", [NL, 4], f32, kind="ExternalOutput")

    with tile.TileContext(nc) as tc:
        with (
            tc.tile_pool(name="const", bufs=1) as cpool,
            tc.tile_pool(name="work", bufs=24) as wpool,
            tc.tile_pool(name="rq", bufs=4) as rqpool,
            tc.tile_pool(name="epi", bufs=2) as epool,
            tc.tile_pool(name="ppro", bufs=4, space="PSUM") as ppro,
            tc.tile_pool(name="phy", bufs=1, space="PSUM") as phy,
            tc.tile_pool(name="pmain", bufs=1, space="PSUM") as pmain,
            tc.tile_pool(name="pdum", bufs=1, space="PSUM") as pdum,
        ):
            # one table load (natural_log_exp_and_others: copy/relu/exp/ln)
            # hidden under the input DMAs instead of mid-epilogue
            nc.scalar.add_instruction(
                mybir.InstLoadActFuncSet(
                    name=nc.get_next_instruction_name(),
                    act_func_set_id=6,
                    engine=mybir.EngineType.Activation,
                    ins=[],
                    outs=[],
                )
            )

            # ---- input DMAs: one slab per k-tile + B3 + mask ----
            sl_t = []
            for k in range(KT):
                t = cpool.tile([KSZY[k], CTOT], fp16, name=f"slab{k}")
                nc.sync.dma_start(
                    out=t, in_=slab[k * 128 : k * 128 + KSZY[k], :]
                )
                sl_t.append(t)
            b3 = cpool.tile([128, 60], fp16, name="b3")
            nc.sync.dma_start(out=b3, in_=b3d[:, :])
            mask = cpool.tile([NH, NH], fp16, name="mask")
            nc.sync.dma_start(out=mask, in_=maskd[:, :])

            xt = [sl_t[k][: KSZ[k], CX : CX + N] for k in range(KT)]
            w1x = [sl_t[k][: KSZ[k], CW1X : CW1X + H + 16] for k in range(KT)]
            w1y = [sl_t[k][: KSZY[k], CW1Y : CW1Y + H + 16] for k in range(KT)]
            yt = [sl_t[k][: KSZY[k], CYT : CYT + NL] for k in range(KT)]
            bshl = [sl_t[k][: KSZ[k], CBSH : CBSH + 128] for k in range(KT)]

            # ---- PE warmup: keep the tensor engine busy from t~0 ----
            dumw = cpool.tile([128, 1], fp16, name="dumw")
            nc.vector.memset(dumw, 0.0)
            dumr = cpool.tile([128, DROWS], fp16, name="dumr")
            nc.vector.memset(dumr, 0.0)
            pd = pdum.tile([1, DROWS], f32, name="pd", tag="pd")

            b2t = cpool.tile([NH, 1], f32, name="b2t")
            nc.vector.memset(b2t, b2val)
            onet = cpool.tile([NH, 1], f32, name="onet")
            nc.vector.memset(onet, 1.0)
            out3 = cpool.tile([NL, 4], f32, name="out3")
            nc.vector.memset(out3, 0.0)

            def dummies(n):
                for _ in range(n):
                    nc.tensor.matmul(pd, lhsT=dumw, rhs=dumr,
                                     start=True, stop=True)

            dummies(NWARM)

            # ---- prologue, k-batch ordered: hy then hx per arriving slab ----
            pyall = phy.tile([128, 4 * NL], f32, name="pyall", tag="py")
            ph = [
                ppro.tile([MSZ[m], N], f32, name=f"ph{m}", tag="pp")
                for m in range(KT)
            ]
            # hx matmuls k-batched (one batch per arriving slab, no stalls;
            # the 4 ph banks are distinct so group interleaving is safe)
            for k in range(KT):
                for m in range(KT):
                    msl = slice(m * 128, m * 128 + MSZ[m])
                    nc.tensor.matmul(
                        ph[m], lhsT=w1x[k][:, msl], rhs=xt[k],
                        start=(k == 0), stop=(k == KT - 1),
                    )
            # hx tiles to fp16 (DVE/ACT split, overlaps the hy matmuls)
            hx = []
            for m in range(KT):
                hxm = cpool.tile([MSZ[m], N], fp16, name=f"hx{m}")
                if m % 2 == 0:
                    nc.vector.tensor_copy(out=hxm, in_=ph[m])
                else:
                    nc.scalar.activation(
                        out=hxm, in_=ph[m], func=AF.Copy, bias=0.0, scale=1.0,
                    )
                hx.append(hxm)
            # hy blocks share one PSUM bank: groups must be sequential per
            # block (same-bank interleaved start/stop corrupts accumulation)
            for m in range(KT):
                msl = slice(m * 128, m * 128 + MSZ[m])
                for k in range(KT):
                    nc.tensor.matmul(
                        pyall[: MSZ[m], m * NL : (m + 1) * NL],
                        lhsT=w1y[k][:, msl], rhs=yt[k],
                        start=(k == 0), stop=(k == KT - 1),
                    )
            # hyb = hy + b1 (one f32 copy)
            hyball = cpool.tile([128, 4 * NL], f32, name="hyball")
            nc.vector.tensor_copy(out=hyball, in_=pyall)

            def hyb(m, i):  # per-partition scalar for H-tile m, row i
                return hyball[: MSZ[m], m * NL + i : m * NL + i + 1]

            dummies(NFILL)  # bridge prologue->main while copies drain

            # ---- main loop: two 32-row halves, quads of 4 rows ----
            def emit_quad(half, q):
                rq = rqpool.tile([128, N], fp16, name="rq", tag="rq")
                rks = []
                for a in range(4):
                    i = half * NH + 4 * q + a
                    for k in range(3):
                        r = wpool.tile([128, N], fp16, name=f"r{k}",
                                       tag=f"r{k}")
                        nc.vector.tensor_scalar(
                            out=r, in0=hx[k], scalar1=hyb(k, i), scalar2=0.0,
                            op0=ALU.add, op1=ALU.max,
                        )
                        rks.append(r)
                    if a == 0:
                        nc.vector.tensor_scalar(
                            out=rq[0:32, :], in0=hx[3],
                            scalar1=hyb(3, i), scalar2=0.0,
                            op0=ALU.add, op1=ALU.max,
                        )
                    else:
                        nc.scalar.activation(
                            out=rq[32 * a : 32 * (a + 1), :], in_=hx[3],
                            func=AF.Relu, bias=hyb(3, i), scale=1.0,
                        )
                for a in range(4):
                    ii = 4 * q + a
                    for k in range(3):
                        nc.tensor.matmul(
                            ps_h[half], lhsT=bshl[k][:, 64 - ii : 96 - ii],
                            rhs=rks[3 * a + k],
                            start=(q == 0 and a == 0 and k == 0), stop=False,
                        )
                nc.tensor.matmul(
                    ps_h[half], lhsT=b3[:, 28 - 4 * q : 60 - 4 * q], rhs=rq,
                    start=False, stop=(q == NH // 4 - 1),
                )

            def emit_epilogue(half):
                osl = slice(half * NH, (half + 1) * NH)
                e2 = epool.tile([NH, N], fp16, name="e2", tag="e2")
                t1s = epool.tile([NH, N], fp16, name="t1s", tag="t1s")
                tmp = epool.tile([NH, NH], fp16, name="tmp", tag="tmp")
                # E = exp(s + b2); rr = row sums of E
                nc.scalar.activation(
                    out=e2, in_=ps_h[half], func=AF.Exp, bias=b2t, scale=1.0,
                    accum_out=out3[osl, 0:1],
                )
                # T1 = log(1 + E); rs = row sums of T1
                nc.scalar.activation(
                    out=t1s, in_=e2, func=AF.Ln, bias=onet, scale=1.0,
                    accum_out=out3[osl, 1:2],
                )
                # ed = diag(E): rotated x puts the diag block at a fixed
                # 32-column window
                nc.vector.tensor_tensor(
                    out=tmp,
                    in0=e2[:, half * NH : (half + 1) * NH], in1=mask,
                    op=ALU.mult,
                )
                nc.vector.reduce_sum(
                    out=out3[osl, 2:3], in_=tmp, axis=mybir.AxisListType.X
                )
                nc.sync.dma_start(out=out_o[osl, :], in_=out3[osl, :])

            ps_h = [
                pmain.tile([NH, N], f32, name=f"ps{h}", tag=f"ps{h}")
                for h in range(2)
            ]
            for q in range(NH // 4):
                emit_quad(0, q)
            for q in range(NH // 4):
                emit_quad(1, q)
                if q == 0:
                    emit_epilogue(0)
            emit_epilogue(1)

    nc.compile()
    return nc


def _make_in_maps(x, y, W1, b1, W2):
    f16 = np.float16
    slab = np.zeros((401, CTOT), f16)
    slab[:D, CX : CX + N] = x.T.astype(f16)
    w1xT = W1[:, :D].T.astype(f16)       # [D(k), H(m)]
    w1yT = W1[:, D:].T.astype(f16)
    slab[:D, CW1X : CW1X + H] = w1xT
    slab[:D, CW1X + H : CW1X + H + 16] = w1xT[:, 384:400]   # m3 dup
    slab[:D, CW1Y : CW1Y + H] = w1yT
    slab[:D, CW1Y + H : CW1Y + H + 16] = w1yT[:, 384:400]
    slab[400, CW1Y : CW1Y + H] = b1.astype(f16)
    slab[400, CW1Y + H : CW1Y + H + 16] = b1[384:400].astype(f16)
    slab[:H, CBSH + 64] = W2[0].astype(f16)
    b3p = np.zeros((128, 60), f16)
    p = np.arange(128)
    val = W2[0, 384 + (p % 32) % 16].astype(f16)
    b3p[p, 28 + p // 32] = np.where(p % 32 < 16, val, 0.0)

    maskp = np.eye(NH, dtype=f16)
    xT = x.T.astype(f16)
    in_maps = []
    for c in range(NCORES):
        s = slab.copy()
        # rotate x columns so core c's diag block lands at columns [0, 64)
        s[:D, CX : CX + N] = np.roll(xT, -c * NL, axis=1)
        s[:D, CYT : CYT + NL] = y[c * NL : (c + 1) * NL, :].T.astype(f16)
        s[400, CYT : CYT + NL] = 1.0
        in_maps.append({"slab": s, "b3d": b3p, "maskd": maskp})
    return in_maps


def _combine(results):
    rr = np.concatenate([r["out_o"][:, 0].astype(np.float64) for r in results])
    rs = np.concatenate([r["out_o"][:, 1].astype(np.float64) for r in results])
    ed = np.concatenate([r["out_o"][:, 2].astype(np.float64) for r in results])
    lse = np.log(np.float64(N) + rr)
    t0 = np.log1p(ed)
    t0_mean = t0.mean()
    lower = t0_mean - (lse.mean() - np.log(np.float64(N)))
    upper = t0_mean - rs.mean() / N
    return np.float32(lower), np.float32(upper)


def kernel(x_samples, y_samples, W1, b1, W2, b2, _trace=False):
    from concourse.bass_utils import run_bass_kernel_spmd

    nc = _build_program(float(np.float32(b2[0])))
    in_maps = _make_in_maps(
        np.asarray(x_samples, np.float32),
        np.asarray(y_samples, np.float32),
        np.asarray(W1, np.float32),
        np.asarray(b1, np.float32),
        np.asarray(W2, np.float32),
    )
    res = run_bass_kernel_spmd(
        nc, in_maps, core_ids=list(range(NCORES)), trace=_trace
    )
    out = _combine(res.results)
    if _trace:
        return out, res
    return out


# revision 17
# speedup vs baseline: 1.5387x; 1.0054x over previous
"""CLUB-NCE loss kernel for 8x Trainium2 NeuronCores (Bass/Tile).

Math (reference):
  hx = x @ W1x.T, hy = y @ W1y.T            [N, H]
  s[i,j]  = W2 . relu(hy[i] + hx[j] + b1) + b2
  T1[i,j] = softplus(s[i,j]); T0[i] = T1[i,i]
  lower = mean(T0) - (mean_i(logsumexp_j(T1[i,:])) - log N)
  upper = mean(T0) - mean(T1)

Sharding: y rows (i axis) split across 8 cores (64 rows each); x and MLP
params replicated. Each core computes its [64, 512] score block and emits
per-row partials (row sum of e^s, row sum of T1, diag e^s). Host combines.

Device design notes:
 - contraction dim k (=H=400) on partitions, tiled [128,128,128,16(+1)].
 - score row i is routed to PSUM partition i via a shifted one-hot
   stationary matrix: bsh[k] is [Pk, 128] with w2[k-chunk] at column 64,
   so lhsT = bsh[k][:, 64-ii : 96-ii] puts w2 in column ii.  All matmuls
   of a 32-row half accumulate into one [32, 512] PSUM bank; rows not
   owned by a matmul get exact +0.
 - the 16-deep k3 tail tiles of 4 consecutive rows are packed into one
   [64, 512] rhs and contracted by a single matmul using a banded
   stationary B3 (B3[p, 28 + p//16] = w2[384 + p%16], window slid by
   4 columns per quad): 13 matmuls per 4 rows instead of 16.
 - b1 is folded into the hy matmul as a virtual k=400 row.
 - PE warmup: dummy matmuls keep the tensor engine continuously busy
   from t~0.7us so it reaches full clock before real work, and filler
   dummies bridge the prologue->main handoff (any idle gap drops the
   PE clock for ~3us).
 - prologue is k-batch ordered (one batch per arriving input slab) so
   the PE never stalls mid-prologue.
 - two 32-row halves; half A's epilogue (Exp + softplus row sums on ACT,
   masked diag on DVE) hides under half B's matmuls.
 - per quad of rows: 12+1 relu tiles split DVE (4x mode) / ACT.
 - inputs arrive in 6 DMAs (one [*,1504] fp16 slab per k-tile + banded
   B3 + fp16 diag mask) to minimize serialized HWDGE occupancy.
 - host finishes: lse_i = log(N + rr_i), t0_i = log(1 + ed_i), means.
"""

import numpy as np

N = 512          # number of samples
D = 400          # feature dim
H = 400          # hidden dim
NCORES = 8
NL = N // NCORES  # 64 y-rows per core
NH = NL // 2      # 32 rows per half
KT = 4            # k tiles
KSZ = [128, 128, 128, 16]    # real k per tile (400 total)
KSZY = [128, 128, 128, 17]   # hy matmul k per tile (incl. bias row)
# consolidated input slabs, split in two pieces per k-tile so the hx
# matmuls can start before the hy-side data arrives:
#   A: x | w1x(+m3 dup)       B: w1y(+m3 dup) | yt | bsh
CX, CW1X = 0, 512
ATOT = 928
CW1Y, CYT, CBSH = 0, 416, 480
BTOT = 608
MSZ = [128, 128, 128, 32]    # H-tile partition sizes (m3 duplicated 16+16)
NWARM = 1         # PE warmup dummy matmuls (anchors the p-state ramp)
NFILL = 0         # PE filler dummies between prologue and main loop
DROWS = 384       # dummy matmul free size


def _build_program(b2val: float, enable_asserts: bool = False):
    import concourse.bacc as bacc
    import concourse.mybir as mybir
    import concourse.tile as tile

    fp16 = mybir.dt.float16
    f32 = mybir.dt.float32
    AF = mybir.ActivationFunctionType
    ALU = mybir.AluOpType

    nc = bacc.Bacc(
        "TRN2",
        target_bir_lowering=False,
        debug=False,
        enable_asserts=enable_asserts,
    )

    slabA = nc.dram_tensor("slabA", [401, ATOT], fp16, kind="ExternalInput")
    slabB = nc.dram_tensor("slabB", [401, BTOT], fp16, kind="ExternalInput")
    b3d = nc.dram_tensor("b3d", [128, 60], fp16, kind="ExternalInput")
    # x columns are rotated per core so the diag block sits at columns
    # [h*32, h*32+32) of half h; the mask is just a [32,32] identity
    maskd = nc.dram_tensor("maskd", [NH, NH], fp16, kind="ExternalInput")
    out_o = nc.dram_tensor("out_o", [NL, 4], f32, kind="ExternalOutput")

    with tile.TileContext(nc) as tc:
        with (
            tc.tile_pool(name="const", bufs=1) as cpool,
            tc.tile_pool(name="work", bufs=24) as wpool,
            tc.tile_pool(name="rq", bufs=4) as rqpool,
            tc.tile_pool(name="epi", bufs=2) as epool,
            tc.tile_pool(name="ppro", bufs=4, space="PSUM") as ppro,
            tc.tile_pool(name="phy", bufs=1, space="PSUM") as phy,
            tc.tile_pool(name="pmain", bufs=1, space="PSUM") as pmain,
            tc.tile_pool(name="pdum", bufs=1, space="PSUM") as pdum,
        ):
            # one table load (natural_log_exp_and_others: copy/relu/exp/ln)
            # hidden under the input DMAs instead of mid-epilogue
            nc.scalar.add_instruction(
                mybir.InstLoadActFuncSet(
                    name=nc.get_next_instruction_name(),
                    act_func_set_id=6,
                    engine=mybir.EngineType.Activation,
                    ins=[],
                    outs=[],
                )
            )

            # ---- input DMAs: A pieces first (feed hx), then B + B3 + mask ----
            sa_t, sb_t = [], []
            for k in range(KT):
                t = cpool.tile([KSZ[k], ATOT], fp16, name=f"slabA{k}")
                nc.sync.dma_start(
                    out=t, in_=slabA[k * 128 : k * 128 + KSZ[k], :]
                )
                sa_t.append(t)
            for k in range(KT):
                t = cpool.tile([KSZY[k], BTOT], fp16, name=f"slabB{k}")
                nc.sync.dma_start(
                    out=t, in_=slabB[k * 128 : k * 128 + KSZY[k], :]
                )
                sb_t.append(t)
            b3 = cpool.tile([128, 60], fp16, name="b3")
            nc.sync.dma_start(out=b3, in_=b3d[:, :])
            mask = cpool.tile([NH, NH], fp16, name="mask")
            nc.sync.dma_start(out=mask, in_=maskd[:, :])

            xt = [sa_t[k][:, CX : CX + N] for k in range(KT)]
            w1x = [sa_t[k][:, CW1X : CW1X + H + 16] for k in range(KT)]
            w1y = [sb_t[k][:, CW1Y : CW1Y + H + 16] for k in range(KT)]
            yt = [sb_t[k][:, CYT : CYT + NL] for k in range(KT)]
            bshl = [sb_t[k][: KSZ[k], CBSH : CBSH + 128] for k in range(KT)]

            # ---- PE warmup: keep the tensor engine busy from t~0 ----
            dumw = cpool.tile([128, 1], fp16, name="dumw")
            nc.vector.memset(dumw, 0.0)
            dumr = cpool.tile([128, DROWS], fp16, name="dumr")
            nc.vector.memset(dumr, 0.0)
            pd = pdum.tile([1, DROWS], f32, name="pd", tag="pd")

            b2t = cpool.tile([NH, 1], f32, name="b2t")
            nc.vector.memset(b2t, b2val)
            onet = cpool.tile([NH, 1], f32, name="onet")
            nc.vector.memset(onet, 1.0)
            out3 = cpool.tile([NL, 4], f32, name="out3")
            nc.vector.memset(out3, 0.0)

            def dummies(n):
                for _ in range(n):
                    nc.tensor.matmul(pd, lhsT=dumw, rhs=dumr,
                                     start=True, stop=True)

            dummies(NWARM)

            # ---- prologue, k-batch ordered: hy then hx per arriving slab ----
            pyall = phy.tile([128, 4 * NL], f32, name="pyall", tag="py")
            ph = [
                ppro.tile([MSZ[m], N], f32, name=f"ph{m}", tag="pp")
                for m in range(KT)
            ]
            # hx matmuls k-batched (one batch per arriving slab, no stalls;
            # the 4 ph banks are distinct so group interleaving is safe)
            for k in range(KT):
                for m in range(KT):
                    msl = slice(m * 128, m * 128 + MSZ[m])
                    nc.tensor.matmul(
                        ph[m], lhsT=w1x[k][:, msl], rhs=xt[k],
                        start=(k == 0), stop=(k == KT - 1),
                    )
            # hx tiles to fp16 (DVE/ACT split, overlaps the hy matmuls)
            hx = []
            for m in range(KT):
                hxm = cpool.tile([MSZ[m], N], fp16, name=f"hx{m}")
                if m % 2 == 0:
                    nc.vector.tensor_copy(out=hxm, in_=ph[m])
                else:
                    nc.scalar.activation(
                        out=hxm, in_=ph[m], func=AF.Copy, bias=0.0, scale=1.0,
                    )
                hx.append(hxm)
            # hy blocks share one PSUM bank: groups must be sequential per
            # block (same-bank interleaved start/stop corrupts accumulation)
            for m in range(KT):
                msl = slice(m * 128, m * 128 + MSZ[m])
                for k in range(KT):
                    nc.tensor.matmul(
                        pyall[: MSZ[m], m * NL : (m + 1) * NL],
                        lhsT=w1y[k][:, msl], rhs=yt[k],
                        start=(k == 0), stop=(k == KT - 1),
                    )
            # hyb = hy + b1 (one f32 copy)
            hyball = cpool.tile([128, 4 * NL], f32, name="hyball")
            nc.vector.tensor_copy(out=hyball, in_=pyall)

            def hyb(m, i):  # per-partition scalar for H-tile m, row i
                return hyball[: MSZ[m], m * NL + i : m * NL + i + 1]

            dummies(NFILL)  # bridge prologue->main while copies drain

            # ---- main loop: two 32-row halves, quads of 4 rows ----
            def emit_quad(half, q):
                rq = rqpool.tile([128, N], fp16, name="rq", tag="rq")
                rks = []
                for a in range(4):
                    i = half * NH + 4 * q + a
                    for k in range(3):
                        r = wpool.tile([128, N], fp16, name=f"r{k}",
                                       tag=f"r{k}")
                        nc.vector.tensor_scalar(
                            out=r, in0=hx[k], scalar1=hyb(k, i), scalar2=0.0,
                            op0=ALU.add, op1=ALU.max,
                        )
                        rks.append(r)
                    if a == 0:
                        nc.vector.tensor_scalar(
                            out=rq[0:32, :], in0=hx[3],
                            scalar1=hyb(3, i), scalar2=0.0,
                            op0=ALU.add, op1=ALU.max,
                        )
                    else:
                        nc.scalar.activation(
                            out=rq[32 * a : 32 * (a + 1), :], in_=hx[3],
                            func=AF.Relu, bias=hyb(3, i), scale=1.0,
                        )
                for a in range(4):
                    ii = 4 * q + a
                    for k in range(3):
                        nc.tensor.matmul(
                            ps_h[half], lhsT=bshl[k][:, 64 - ii : 96 - ii],
                            rhs=rks[3 * a + k],
                            start=(q == 0 and a == 0 and k == 0), stop=False,
                        )
                nc.tensor.matmul(
                    ps_h[half], lhsT=b3[:, 28 - 4 * q : 60 - 4 * q], rhs=rq,
                    start=False, stop=(q == NH // 4 - 1),
                )

            def emit_epilogue(half):
                osl = slice(half * NH, (half + 1) * NH)
                e2 = epool.tile([NH, N], fp16, name="e2", tag="e2")
                t1s = epool.tile([NH, N], fp16, name="t1s", tag="t1s")
                tmp = epool.tile([NH, NH], fp16, name="tmp", tag="tmp")
                # E = exp(s + b2); rr = row sums of E
                nc.scalar.activation(
                    out=e2, in_=ps_h[half], func=AF.Exp, bias=b2t, scale=1.0,
                    accum_out=out3[osl, 0:1],
                )
                # T1 = log(1 + E); rs = row sums of T1
                nc.scalar.activation(
                    out=t1s, in_=e2, func=AF.Ln, bias=onet, scale=1.0,
                    accum_out=out3[osl, 1:2],
                )
                # ed = diag(E): rotated x puts the diag block at a fixed
                # 32-column window
                nc.vector.tensor_tensor(
                    out=tmp,
                    in0=e2[:, half * NH : (half + 1) * NH], in1=mask,
                    op=ALU.mult,
                )
                nc.vector.reduce_sum(
                    out=out3[osl, 2:3], in_=tmp, axis=mybir.AxisListType.X
                )
                nc.sync.dma_start(out=out_o[osl, :], in_=out3[osl, :])

            ps_h = [
                pmain.tile([NH, N], f32, name=f"ps{h}", tag=f"ps{h}")
                for h in range(2)
            ]
            for q in range(NH // 4):
                emit_quad(0, q)
            for q in range(NH // 4):
                emit_quad(1, q)
                if q == 0:
                    emit_epilogue(0)
            emit_epilogue(1)

    nc.compile()
    return nc


def _make_in_maps(x, y, W1, b1, W2):
    f16 = np.float16
    slabA = np.zeros((401, ATOT), f16)
    slabB = np.zeros((401, BTOT), f16)
    w1xT = W1[:, :D].T.astype(f16)       # [D(k), H(m)]
    w1yT = W1[:, D:].T.astype(f16)
    slabA[:D, CW1X : CW1X + H] = w1xT
    slabA[:D, CW1X + H : CW1X + H + 16] = w1xT[:, 384:400]   # m3 dup
    slabB[:D, CW1Y : CW1Y + H] = w1yT
    slabB[:D, CW1Y + H : CW1Y + H + 16] = w1yT[:, 384:400]
    slabB[400, CW1Y : CW1Y + H] = b1.astype(f16)
    slabB[400, CW1Y + H : CW1Y + H + 16] = b1[384:400].astype(f16)
    slabB[:H, CBSH + 64] = W2[0].astype(f16)
    b3p = np.zeros((128, 60), f16)
    p = np.arange(128)
    val = W2[0, 384 + (p % 32) % 16].astype(f16)
    b3p[p, 28 + p // 32] = np.where(p % 32 < 16, val, 0.0)

    maskp = np.eye(NH, dtype=f16)
    xT = x.T.astype(f16)
    in_maps = []
    for c in range(NCORES):
        sa = slabA.copy()
        # rotate x columns so core c's diag block lands at columns [0, 64)
        sa[:D, CX : CX + N] = np.roll(xT, -c * NL, axis=1)
        sb = slabB.copy()
        sb[:D, CYT : CYT + NL] = y[c * NL : (c + 1) * NL, :].T.astype(f16)
        sb[400, CYT : CYT + NL] = 1.0
        in_maps.append({"slabA": sa, "slabB": sb, "b3d": b3p, "maskd": maskp})
    return in_maps


def _combine(results):
    rr = np.concatenate([r["out_o"][:, 0].astype(np.float64) for r in results])
    rs = np.concatenate([r["out_o"][:, 1].astype(np.float64) for r in results])
    ed = np.concatenate([r["out_o"][:, 2].astype(np.float64) for r in results])
    lse = np.log(np.float64(N) + rr)
    t0 = np.log1p(ed)
    t0_mean = t0.mean()
    lower = t0_mean - (lse.mean() - np.log(np.float64(N)))
    upper = t0_mean - rs.mean() / N
    return np.float32(lower), np.float32(upper)


def kernel(x_samples, y_samples, W1, b1, W2, b2, _trace=False):
    from concourse.bass_utils import run_bass_kernel_spmd

    nc = _build_program(float(np.float32(b2[0])))
    in_maps = _make_in_maps(
        np.asarray(x_samples, np.float32),
        np.asarray(y_samples, np.float32),
        np.asarray(W1, np.float32),
        np.asarray(b1, np.float32),
        np.asarray(W2, np.float32),
    )
    res = run_bass_kernel_spmd(
        nc, in_maps, core_ids=list(range(NCORES)), trace=_trace
    )
    out = _combine(res.results)
    if _trace:
        return out, res
    return out
